# revision 9
# baseline (speedup 1.0000x reference)
"""Multi-head attention (B=2, N=4096, D=512, H=8) on 8 TRN2 NeuronCores.

Sharding: head-parallel (core d owns head d, both batches). v2 layout:
  - Token-major streamed xT DMA: fine-grained pieces for the first two
    512-token blocks so kT[0]/qT[0] matmuls start ~5us in; first exp ~10us.
  - JIT production: kT/qT/v tiles are produced inside the attention loop's
    PE slack, gated on their xT blocks' DMA arrival.
  - PV matmuls run on a quota-paced lag behind the exp stream (per-qc pair
    quotas) so the production-heavy first q-chunks don't starve ScalarE,
    which is the bottleneck (33.5M exps/core ~ 287us busy).
  - Output A2A split into 3 pipelined pieces (after qc3 / qc5 / qc7) with
    token ownership striped across completion order; unpack DMAs + their
    waits ride the idle GpSimd queue (never block the Sync queue), proj
    subtiles for pieces 0-1 are interleaved into late attention.
  - fp16 everywhere off-PSUM (better mantissa than bf16; same speed).
Host side only transposes/casts inputs and scatters the 8 output slices.
"""

from collections import deque
from contextlib import ExitStack

import numpy as np

N_CORES = 8
B, N, D = 2, 4096, 512
H, HD = 8, 64
T = B * N              # 8192 flattened tokens
TS = T // N_CORES      # 1024 tokens output slice per core
SCALE = HD ** -0.5
KC = D // 128          # 4 contraction chunks of the model dim
NKT = N // 128         # 32 k-token tiles per batch
QC = 512               # q-chunk processed per accumulator
NQC = N // QC          # 8 q-chunks per batch

F16 = np.float16

# PV emission quota per qc, in (kt, both-batch) pair units; sums to 256.
# Lag after each qc: 16, 22, 18, 10, 6, 4, 2, 0 — smooths the JIT
# production load of qc0-2 into later windows, keeps a small lag at late
# boundaries (so acc-ring recycling behind collective-delayed norm DMAs
# never blocks the next qc's scores), drains before the tail.
PV_QUOTA = [16, 26, 36, 40, 36, 34, 34, 34]

_COMPILED = {}


def _patch_tile_drain():
    """The walrus build in this container caps sync waits at 1 per
    instruction (2 for EventSemaphore), but TileContext._drain_and_barrier
    puts every live proc's final wait on a single Drain, which fails
    codegen with 'Too many sync wait commands'. Re-emit those waits as
    individual wait_ge instructions before the drain."""
    import concourse.mybir as mybir
    import concourse.tile as tile
    from concourse.bass_types import SemaphoreHandle
    from concourse.vector_clock import ScopedClock

    if getattr(tile.TileContext, "_drain_patch_installed", False):
        return

    def _drain_and_barrier(self, tick_clock, wait_clock):
        probe = mybir.InstNoOp(name=f"drain-probe-{self.nc.next_id()}", ins=[], outs=[])
        probe.engine = mybir.EngineType.SP
        wait_clock.add_sem_waits(probe, ScopedClock({None: tick_clock.global_clock}))
        waits = probe.sync_info.on_wait if probe.sync_info is not None else []
        for w in waits:
            assert w.wait_mode == "sem-ge-imm", w
            self.nc.sync.wait_ge(SemaphoreHandle(w.ant_name, w.id), w.wait_value)
        self.nc.sync.drain()

        self.nc.all_engine_barrier()
        assert self.sems is not None
        popped = self.nc._tile_sem_poison_stack.pop()
        assert popped is self._sem_poison
        self.nc.clear_and_free_semaphores(list(self.sems.allocated().values()))
        self.nc.all_engine_barrier()

    tile.TileContext._drain_and_barrier = _drain_and_barrier
    tile.TileContext._drain_patch_installed = True


def _patch_multiwait_split():
    """This walrus build rejects instructions with more than one sync wait
    ('Too many sync wait commands'), but Tile's wait assigner can emit
    several waits on one instruction. Post-process the serialized BIR:
    move excess waits onto single-wait EventSemaphore instructions inserted
    just before the owning instruction (same engine => executes in order)."""
    import json

    import concourse.bass as bass

    if getattr(bass.Bass, "_multiwait_patch_installed", False):
        return
    orig = bass.Bass.to_json_bytes

    def to_json_bytes(self, *a, **kw):
        data = json.loads(orig(self, *a, **kw))
        n_split = 0
        for fn in data.get("functions", []):
            for bb in fn.get("blocks", []):
                insts = bb.get("instructions")
                if not insts:
                    continue
                out = []
                for inst in insts:
                    si = inst.get("sync_info")
                    ow = (si or {}).get("on_wait") or []
                    if len(ow) > 1:
                        for i, w in enumerate(ow[:-1]):
                            out.append({
                                "debug": inst.get("debug", 0),
                                "engine": inst["engine"],
                                "ins": [],
                                "outs": [],
                                "name": f"{inst['name']}-esw{i}",
                                "opcode": "EventSemaphore",
                                "sync_info": {"on_update": [], "on_wait": [w]},
                            })
                            n_split += 1
                        si["on_wait"] = [ow[-1]]
                    out.append(inst)
                bb["instructions"] = out
        return json.dumps(data).encode()

    bass.Bass.to_json_bytes = to_json_bytes
    bass.Bass._multiwait_patch_installed = True


def _build():
    import concourse.bass as bass
    import concourse.mybir as mybir
    import concourse.tile as tile

    _patch_tile_drain()
    _patch_multiwait_split()
    dt = mybir.dt
    nc = bass.Bass(num_devices=N_CORES)

    xT_ext = nc.declare_dram_parameter("xT", [D, T], dt.float16, isOutput=False)
    wqT_ext = nc.declare_dram_parameter("wqT", [D, HD], dt.float16, isOutput=False)
    wkT_ext = nc.declare_dram_parameter("wkT", [D, HD], dt.float16, isOutput=False)
    wvT_ext = nc.declare_dram_parameter("wvT", [D, HD], dt.float16, isOutput=False)
    wpT_ext = nc.declare_dram_parameter("wpT", [D, D], dt.float16, isOutput=False)
    bias_ext = nc.declare_dram_parameter("bias", [1, D], dt.float16, isOutput=False)
    out_ext = nc.declare_dram_parameter("out", [TS, D], dt.float16, isOutput=True)

    with tile.TileContext(nc) as tc, ExitStack() as ctx:
        singles = ctx.enter_context(tc.tile_pool(name="singles", bufs=1))
        dram = ctx.enter_context(tc.tile_pool(name="dram", bufs=4, space="DRAM"))
        cpool = ctx.enter_context(tc.tile_pool(name="cpool", bufs=4))

        # ---------- persistent SBUF ----------
        xT_k = [
            singles.tile([128, T], dt.float16, tag=f"xT{k}", name=f"xT{k}")
            for k in range(KC)
        ]
        wqT_sb = singles.tile([128, KC, HD], dt.float16)
        wkT_sb = singles.tile([128, KC, HD], dt.float16)
        wvT_sb = singles.tile([128, KC, HD], dt.float16)
        wpT_sb = singles.tile([128, KC, D], dt.float16)
        bias_sb = singles.tile([1, D], dt.float16)
        ones_sb = singles.tile([1, 128], dt.float16)
        ones65f = singles.tile([1, 1 + HD], dt.float32)
        kT_t = [
            singles.tile([128, 512], dt.float16, tag=f"kT{i}", name=f"kT{i}")
            for i in range(NQC)
        ]
        qT_t = [
            singles.tile([128, 512], dt.float16, tag=f"qT{i}", name=f"qT{i}")
            for i in range(NQC)
        ]
        vp_t = [
            singles.tile([128, 1 + HD], dt.float16, tag=f"vp{t}", name=f"vp{t}")
            for t in range(T // 128)
        ]
        outTall_sb = singles.tile([128, KC, TS], dt.float16)

        # A2A pieces: P0 after qc3 (512 tok/dest), P1 after qc5 (256),
        # P2 after qc7 (256). dim0 = dest core for in, src core for out.
        a2a_in = [
            dram.tile([N_CORES, HD, n], dt.float16, tag=f"a2a_in{p}", bufs=1,
                      name=f"a2a_in{p}")
            for p, n in ((0, 512), (1, 256), (2, 128), (3, 128))
        ]
        a2a_out = [
            dram.tile([N_CORES, HD, n], dt.float16, tag=f"a2a_out{p}", bufs=1,
                      name=f"a2a_out{p}")
            for p, n in ((0, 512), (1, 256), (2, 128), (3, 128))
        ]
        # outTall col ranges fed by each piece
        piece_cols = [(0, 512), (512, 768), (768, 896), (896, 1024)]

        # exp table warm-up: a dummy activation with no data deps loads the
        # ACT table set (~2.7us) during the DMA window instead of at the
        # first real exp.
        warm_in = singles.tile([1, 16], dt.float32)
        warm_out = singles.tile([1, 16], dt.float16)
        nc.vector.memset(warm_in[:], 0.0)
        nc.scalar.activation(
            warm_out[:], warm_in[:], mybir.ActivationFunctionType.Exp
        )

        # ---------- weights + constants ----------
        # qk weights first (they gate the first production chains), then
        # the first two xT blocks, then the rest of the weights.
        for w_sb, w_ext in ((wkT_sb, wkT_ext), (wqT_sb, wqT_ext)):
            nc.sync.dma_start(
                out=w_sb[:], in_=w_ext[:].rearrange("(k p) c -> p k c", p=128)
            )
        nc.vector.memset(ones_sb[:], 1.0)
        nc.vector.memset(ones65f[:], 1.0)
        for t in range(T // 128):
            nc.vector.memset(vp_t[t][:, 0:1], 1.0)

        # ---------- xT DMA, token-major ----------
        # Issue instructions cost ~0.6us each on an engine queue, so split
        # them across the Sync and (idle) GpSimd queues to double the
        # issue rate. blocks 0-1: half-partition pieces (64KB) so block 0
        # lands ~4us after issue. blocks 2-7: one DMA per (blk, k)
        # covering both batches via a strided free dim.
        for blk in range(2):
            for b in range(B):
                for k in range(KC):
                    c0 = b * N + blk * 512
                    for ph in range(2):
                        p0 = ph * 64
                        nc.sync.dma_start(
                            out=xT_k[k][p0:p0 + 64, c0:c0 + 512],
                            in_=xT_ext[k * 128 + p0:k * 128 + p0 + 64,
                                       c0:c0 + 512],
                        )
        nc.sync.dma_start(
            out=wvT_sb[:], in_=wvT_ext[:].rearrange("(k p) c -> p k c", p=128)
        )
        nc.sync.dma_start(
            out=wpT_sb[:], in_=wpT_ext[:].rearrange("(k p) c -> p k c", p=128)
        )
        nc.sync.dma_start(out=bias_sb[:], in_=bias_ext[:])
        for blk in range(2, NQC):
            for k in range(KC):
                eng = nc.sync if (blk * KC + k) % 2 == 0 else nc.gpsimd
                t_ap = xT_k[k][:]
                e_ap = xT_ext[k * 128:(k + 1) * 128, :]
                off = blk * 512
                eng.dma_start(
                    out=bass.AP(
                        tensor=t_ap.tensor, offset=t_ap.offset + off,
                        ap=[list(t_ap.ap[0]), [N, B], [1, 512]],
                    ),
                    in_=bass.AP(
                        tensor=e_ap.tensor, offset=e_ap.offset + off,
                        ap=[list(e_ap.ap[0]), [N, B], [1, 512]],
                    ),
                )

        # ---------- attention ----------
        with (
            tc.tile_pool(name="pst", bufs=2, space="PSUM") as pst,
            tc.tile_pool(name="pacc", bufs=4, space="PSUM") as pacc,
        ):
            def produce_kq(w_sb, dst, blk, pname):
                ps = pacc.tile([128, 512], dt.float32, tag="acc", name=f"{pname}{blk}")
                for k in range(KC):
                    nc.tensor.matmul(
                        ps[0:64, :],
                        lhsT=w_sb[:, k, :],
                        rhs=xT_k[k][:, blk * 512:(blk + 1) * 512],
                        start=(k == 0), stop=(k == KC - 1),
                        tile_position=(0, 0),
                    )
                    nc.tensor.matmul(
                        ps[64:128, :],
                        lhsT=w_sb[:, k, :],
                        rhs=xT_k[k][:, N + blk * 512:N + (blk + 1) * 512],
                        start=(k == 0), stop=(k == KC - 1),
                        tile_position=(0, 64),
                    )
                nc.vector.tensor_copy(dst[:], ps[:])

            def produce_v(t):
                pv = pacc.tile([128, HD], dt.float32, tag="acc", name=f"pv{t}")
                for k in range(KC):
                    nc.tensor.matmul(
                        pv[:],
                        lhsT=xT_k[k][:, t * 128:(t + 1) * 128],
                        rhs=wvT_sb[:, k, :],
                        start=(k == 0), stop=(k == KC - 1),
                    )
                nc.vector.tensor_copy(vp_t[t][:, 1:1 + HD], pv[:])

            def emit_scores_exp(qc, kt):
                st = pst.tile([128, B, QC], dt.float32, tag="st",
                              name=f"st{qc}_{kt}")
                for pair in range(B):
                    pb = pair * 64
                    lhs_k = kT_t[kt // 4][pb:pb + 64,
                                          (kt % 4) * 128:(kt % 4) * 128 + 128]
                    nc.tensor.matmul(
                        st[:, pair, :],
                        lhsT=lhs_k,
                        rhs=qT_t[qc][pb:pb + 64, :],
                        start=True,
                        stop=True,
                        tile_position=(pb, 0),
                    )
                e = cpool.tile([128, B, QC], dt.float16, tag="e", bufs=26,
                               name=f"e{qc}_{kt}")
                nc.scalar.activation(
                    e[:], st[:], mybir.ActivationFunctionType.Exp, scale=SCALE
                )
                return e

            # normalization: reciprocal of the denominator row, partition
            # broadcast, fused scale-multiply, A2A slice scatter. Pairs are
            # interleaved to halve the serial latency. qc0-6 broadcast via
            # a DRAM bounce (off every engine); qc7 — the exposed tail —
            # broadcasts via a K=1 PE matmul instead (no DMA hops; the
            # PSUM ring is free by then).
            def emit_norm(qc, accs):
                rvs = []
                for pair in range(B):
                    rvec = cpool.tile([1, QC], dt.float32, tag="rvec",
                                      name=f"rv{qc}_{pair}")
                    nc.vector.reciprocal(rvec[:], accs[pair][0:1, :])
                    rvs.append(rvec)
                bcs = []
                if qc < NQC - 1:
                    rds = []
                    for pair in range(B):
                        rdram = dram.tile([1, QC], dt.float32, tag="rdram")
                        nc.sync.dma_start(out=rdram[:], in_=rvs[pair][:])
                        rds.append(rdram)
                    for pair in range(B):
                        bcast = cpool.tile([1 + HD, QC], dt.float32,
                                           tag="bcast")
                        r_ap = rds[pair][:]
                        nc.sync.dma_start(
                            out=bcast[:],
                            in_=bass.AP(
                                tensor=r_ap.tensor, offset=r_ap.offset,
                                ap=[[0, 1 + HD]] + list(r_ap.ap[1:]),
                            ),
                        )
                        bcs.append((bcast, accs[pair]))
                else:
                    for pair in range(B):
                        bc = pacc.tile([1 + HD, QC], dt.float32, tag="acc",
                                       name=f"bc{qc}_{pair}")
                        nc.tensor.matmul(
                            bc[:], lhsT=ones65f[:], rhs=rvs[pair][:],
                            start=True, stop=True,
                        )
                        accS = cpool.tile([1 + HD, QC], dt.float32,
                                          tag="bcast", name=f"aS{qc}_{pair}")
                        nc.vector.tensor_copy(accS[:], accs[pair][:])
                        bcs.append((bc, accS))
                for pair in range(B):
                    src_a, src_b = bcs[pair]
                    outTn = cpool.tile([1 + HD, QC], dt.float16, tag="outTn",
                                       name=f"oTn{qc}_{pair}")
                    nc.vector.tensor_mul(outTn[:], src_b[:], src_a[:])
                    # scatter into the A2A piece buffers
                    if qc < 4:
                        j0 = qc * 4 + pair * 2
                        for h in range(2):
                            j = j0 + h
                            dest, pos = j % N_CORES, j // N_CORES
                            nc.sync.dma_start(
                                out=a2a_in[0][dest][:,
                                                    pos * 256:pos * 256 + 256],
                                in_=outTn[1:1 + HD, h * 256:h * 256 + 256],
                            )
                    elif qc < 6:
                        for h in range(2):
                            j = (qc - 4) * 4 + pair * 2 + h
                            nc.sync.dma_start(
                                out=a2a_in[1][j][:, :],
                                in_=outTn[1:1 + HD, h * 256:h * 256 + 256],
                            )
                    else:
                        # split the last pieces' writes across two queues:
                        # 8 issue instructions on one queue cost ~4.4us of
                        # exposed tail latency otherwise
                        piece = 2 if qc == 6 else 3
                        eng = nc.sync if pair == 0 else nc.gpsimd
                        for h4 in range(4):
                            j = pair * 4 + h4
                            eng.dma_start(
                                out=a2a_in[piece][j][:, :],
                                in_=outTn[1:1 + HD, h4 * 128:h4 * 128 + 128],
                            )
                if qc == 3:
                    trigger_piece(0)
                elif qc == 5:
                    trigger_piece(1)
                elif qc == 6:
                    trigger_piece(2)
                elif qc == 7:
                    trigger_piece(3)

            def trigger_piece(p):
                nc.gpsimd.collective_compute(
                    "AllToAll",
                    mybir.AluOpType.bypass,
                    replica_groups=[list(range(N_CORES))],
                    ins=[a2a_in[p].opt()],
                    outs=[a2a_out[p].opt()],
                )

            def unpack_piece(p):
                lo, hi = piece_cols[p]
                for k in range(KC):
                    nc.gpsimd.dma_start(
                        out=outTall_sb[:, k, lo:hi],
                        in_=a2a_out[p][2 * k:2 * k + 2].rearrange(
                            "a d n -> (a d) n"),
                    )

            def proj_subtile(ts_i):
                yp = pacc.tile([128, D], dt.float32, tag="acc", name=f"yp{ts_i}")
                for k in range(KC):
                    nc.tensor.matmul(
                        yp[:],
                        lhsT=outTall_sb[:, k, ts_i * 128:(ts_i + 1) * 128],
                        rhs=wpT_sb[:, k, :],
                        start=(k == 0),
                        stop=False,
                    )
                nc.tensor.matmul(
                    yp[:],
                    lhsT=ones_sb[:],
                    rhs=bias_sb[:],
                    start=False,
                    stop=True,
                )
                y_sb = cpool.tile([128, D], dt.float16, tag="y", name=f"y{ts_i}")
                nc.vector.tensor_copy(y_sb[:], yp[:])
                # split across two queues so the final output lands faster
                for ph in range(2):
                    p0 = ph * 64
                    nc.sync.dma_start(
                        out=out_ext[ts_i * 128 + p0:ts_i * 128 + p0 + 64, :],
                        in_=y_sb[p0:p0 + 64, :],
                    )

            # pre-loop production (gated on block-0/1 DMA + weights)
            produce_kq(wkT_sb, kT_t[0], 0, "k")
            produce_kq(wqT_sb, qT_t[0], 0, "q")
            produce_kq(wkT_sb, kT_t[1], 1, "k")

            # JIT production schedule: extras[(qc, kt)] emitted right after
            # that slot's exp.
            extras = {}
            for blk in range(2, NQC):
                extras.setdefault((0, 4 * blk - 6), []).append(
                    (lambda b: lambda: produce_kq(wkT_sb, kT_t[b], b, "k"))(blk))
            extras.setdefault((0, 26), []).append(
                lambda: produce_kq(wqT_sb, qT_t[1], 1, "q"))
            for i, n_ in enumerate(range(2, 6)):
                extras.setdefault((1, 2 + 8 * i), []).append(
                    (lambda m: lambda: produce_kq(wqT_sb, qT_t[m], m, "q"))(n_))
            for i, n_ in enumerate(range(6, NQC)):
                extras.setdefault((2, 2 + 8 * i), []).append(
                    (lambda m: lambda: produce_kq(wqT_sb, qT_t[m], m, "q"))(n_))

            # PV stream state
            pending = deque()   # (qc, kt, e_tile)
            acc_of = {}         # qc -> [acc_b0, acc_b1]
            v_done = set()
            pv_emitted = 0

            def emit_pv_pair():
                qc, kt, e = pending.popleft()
                if qc not in acc_of:
                    acc_of[qc] = [
                        pacc.tile([1 + HD, QC], dt.float32, tag="acc",
                                  name=f"acc{qc}_{p}")
                        for p in range(B)
                    ]
                for pair in range(B):
                    vidx = pair * NKT + kt
                    if vidx not in v_done:
                        produce_v(vidx)
                        v_done.add(vidx)
                    nc.tensor.matmul(
                        acc_of[qc][pair][:, :],
                        lhsT=vp_t[vidx][:],
                        rhs=e[:, pair, :],
                        start=(kt == 0),
                        stop=(kt == NKT - 1),
                    )
                if kt == NKT - 1:
                    emit_norm(qc, acc_of.pop(qc))

            for qc in range(NQC):
                base = pv_emitted
                for kt in range(NKT):
                    e = emit_scores_exp(qc, kt)
                    pending.append((qc, kt, e))
                    for fn in extras.get((qc, kt), ()):
                        fn()
                    # pace PV emission: quota spread evenly across the qc
                    goal = base + (PV_QUOTA[qc] * (kt + 1)) // NKT
                    while pv_emitted < goal and pending:
                        emit_pv_pair()
                        pv_emitted += 1
                    # late-attention overlap of unpack + proj for the
                    # already-landed pieces: one subtile every other slot so
                    # the PE burst never outruns the pst-buffered exp stream
                    if qc == 7:
                        # start at kt=4: qc6's accs (norm runs ~2 slots in
                        # due to the boundary lag) must free their ring
                        # slots before proj's PSUM allocations
                        if 4 <= kt <= 14 and kt % 2 == 0:
                            proj_subtile(kt // 2 - 2)
                        elif kt == 24:
                            unpack_piece(2)
                            proj_subtile(6)
                # P0 and P1 are both complete well before qc6 ends
                if qc == 6:
                    unpack_piece(0)
                    unpack_piece(1)

            while pending:
                emit_pv_pair()
                pv_emitted += 1

            # keep the PE HAM-warm through the last A2A flight so the final
            # proj subtile runs at full clock
            scratch = pacc.tile([1, D], dt.float32, tag="acc", name="scratch")
            for _ in range(36):
                nc.tensor.matmul(
                    scratch[:], lhsT=ones_sb[:, 0:1], rhs=bias_sb[:],
                    start=True, stop=True,
                )
            unpack_piece(3)
            proj_subtile(7)

    return nc


def _get_nc():
    if "nc" not in _COMPILED:
        _COMPILED["nc"] = _build()
    return _COMPILED["nc"]


def _seg_token(piece, j):
    """Map (piece, subchunk index) -> (batch, token start, length)."""
    if piece == 0:
        qc = j // 4
        rem = j % 4
        return rem // 2, qc * 512 + (rem % 2) * 256, 256
    if piece == 1:
        qc = 4 + j // 4
        rem = j % 4
        return rem // 2, qc * 512 + (rem % 2) * 256, 256
    qc = 6 if piece == 2 else 7
    return j // 4, qc * 512 + (j % 4) * 128, 128


def kernel(x, w_qkv, w_proj, b_proj):
    from concourse.bass_utils import run_bass_kernel_spmd

    x = np.asarray(x, dtype=np.float32)
    w_qkv = np.asarray(w_qkv, dtype=np.float32)
    w_proj = np.asarray(w_proj, dtype=np.float32)
    b_proj = np.asarray(b_proj, dtype=np.float32)

    xT = np.ascontiguousarray(x.transpose(2, 0, 1).reshape(D, T)).astype(F16)
    wpT = np.ascontiguousarray(w_proj.T).astype(F16)
    bias = b_proj.reshape(1, D).astype(F16)

    in_maps = []
    for d in range(N_CORES):
        wq = w_qkv[0 * D + d * HD: 0 * D + (d + 1) * HD, :]
        wk = w_qkv[1 * D + d * HD: 1 * D + (d + 1) * HD, :]
        wv = w_qkv[2 * D + d * HD: 2 * D + (d + 1) * HD, :]
        in_maps.append({
            "xT": xT,
            "wqT": np.ascontiguousarray(wq.T).astype(F16),
            "wkT": np.ascontiguousarray(wk.T).astype(F16),
            "wvT": np.ascontiguousarray(wv.T).astype(F16),
            "wpT": wpT,
            "bias": bias,
        })

    nc = _get_nc()
    res = run_bass_kernel_spmd(nc, in_maps, core_ids=list(range(N_CORES)))

    y = np.empty((B, N, D), dtype=np.float32)
    for s in range(N_CORES):
        r = np.asarray(res.results[s]["out"], dtype=np.float32)
        segs = [(0, s), (0, s + 8), (1, s), (2, s), (3, s)]
        row = 0
        for piece, j in segs:
            b, t0, ln = _seg_token(piece, j)
            y[b, t0:t0 + ln, :] = r[row:row + ln, :]
            row += ln
    return y


# revision 10
# speedup vs baseline: 1.0359x; 1.0359x over previous
"""Multi-head attention (B=2, N=4096, D=512, H=8) on 8 TRN2 NeuronCores.

Sharding: head-parallel (core d owns head d, both batches). v2 layout:
  - Token-major streamed xT DMA: fine-grained pieces for the first two
    512-token blocks so kT[0]/qT[0] matmuls start ~5us in; first exp ~10us.
  - JIT production: kT/qT/v tiles are produced inside the attention loop's
    PE slack, gated on their xT blocks' DMA arrival.
  - PV matmuls run on a quota-paced lag behind the exp stream (per-qc pair
    quotas) so the production-heavy first q-chunks don't starve ScalarE,
    which is the bottleneck (33.5M exps/core ~ 287us busy).
  - Output A2A split into 3 pipelined pieces (after qc3 / qc5 / qc7) with
    token ownership striped across completion order; unpack DMAs + their
    waits ride the idle GpSimd queue (never block the Sync queue), proj
    subtiles for pieces 0-1 are interleaved into late attention.
  - fp16 everywhere off-PSUM (better mantissa than bf16; same speed).
Host side only transposes/casts inputs and scatters the 8 output slices.
"""

from collections import deque
from contextlib import ExitStack

import numpy as np

N_CORES = 8
B, N, D = 2, 4096, 512
H, HD = 8, 64
T = B * N              # 8192 flattened tokens
TS = T // N_CORES      # 1024 tokens output slice per core
SCALE = HD ** -0.5
KC = D // 128          # 4 contraction chunks of the model dim
NKT = N // 128         # 32 k-token tiles per batch
QC = 512               # q-chunk processed per accumulator
NQC = N // QC          # 8 q-chunks per batch

F16 = np.float16

# PV emission quota per qc, in (kt, both-batch) pair units; sums to 256.
# Lag after each qc: 16, 22, 18, 10, 6, 4, 2, 0 — smooths the JIT
# production load of qc0-2 into later windows, keeps a small lag at late
# boundaries (so acc-ring recycling behind collective-delayed norm DMAs
# never blocks the next qc's scores), drains before the tail.
PV_QUOTA = [16, 26, 36, 40, 36, 34, 34, 34]

_COMPILED = {}


def _patch_tile_drain():
    """The walrus build in this container caps sync waits at 1 per
    instruction (2 for EventSemaphore), but TileContext._drain_and_barrier
    puts every live proc's final wait on a single Drain, which fails
    codegen with 'Too many sync wait commands'. Re-emit those waits as
    individual wait_ge instructions before the drain."""
    import concourse.mybir as mybir
    import concourse.tile as tile
    from concourse.bass_types import SemaphoreHandle
    from concourse.vector_clock import ScopedClock

    if getattr(tile.TileContext, "_drain_patch_installed", False):
        return

    def _drain_and_barrier(self, tick_clock, wait_clock):
        probe = mybir.InstNoOp(name=f"drain-probe-{self.nc.next_id()}", ins=[], outs=[])
        probe.engine = mybir.EngineType.SP
        wait_clock.add_sem_waits(probe, ScopedClock({None: tick_clock.global_clock}))
        waits = probe.sync_info.on_wait if probe.sync_info is not None else []
        for w in waits:
            assert w.wait_mode == "sem-ge-imm", w
            self.nc.sync.wait_ge(SemaphoreHandle(w.ant_name, w.id), w.wait_value)
        self.nc.sync.drain()

        self.nc.all_engine_barrier()
        assert self.sems is not None
        popped = self.nc._tile_sem_poison_stack.pop()
        assert popped is self._sem_poison
        self.nc.clear_and_free_semaphores(list(self.sems.allocated().values()))
        self.nc.all_engine_barrier()

    tile.TileContext._drain_and_barrier = _drain_and_barrier
    tile.TileContext._drain_patch_installed = True


def _patch_multiwait_split():
    """This walrus build rejects instructions with more than one sync wait
    ('Too many sync wait commands'), but Tile's wait assigner can emit
    several waits on one instruction. Post-process the serialized BIR:
    move excess waits onto single-wait EventSemaphore instructions inserted
    just before the owning instruction (same engine => executes in order)."""
    import json

    import concourse.bass as bass

    if getattr(bass.Bass, "_multiwait_patch_installed", False):
        return
    orig = bass.Bass.to_json_bytes

    def to_json_bytes(self, *a, **kw):
        data = json.loads(orig(self, *a, **kw))
        n_split = 0
        for fn in data.get("functions", []):
            for bb in fn.get("blocks", []):
                insts = bb.get("instructions")
                if not insts:
                    continue
                out = []
                for inst in insts:
                    si = inst.get("sync_info")
                    ow = (si or {}).get("on_wait") or []
                    if len(ow) > 1:
                        for i, w in enumerate(ow[:-1]):
                            out.append({
                                "debug": inst.get("debug", 0),
                                "engine": inst["engine"],
                                "ins": [],
                                "outs": [],
                                "name": f"{inst['name']}-esw{i}",
                                "opcode": "EventSemaphore",
                                "sync_info": {"on_update": [], "on_wait": [w]},
                            })
                            n_split += 1
                        si["on_wait"] = [ow[-1]]
                    out.append(inst)
                bb["instructions"] = out
        return json.dumps(data).encode()

    bass.Bass.to_json_bytes = to_json_bytes
    bass.Bass._multiwait_patch_installed = True


def _build():
    import concourse.bass as bass
    import concourse.mybir as mybir
    import concourse.tile as tile

    _patch_tile_drain()
    _patch_multiwait_split()
    dt = mybir.dt
    nc = bass.Bass(num_devices=N_CORES)

    xT_ext = nc.declare_dram_parameter("xT", [D, T], dt.float16, isOutput=False)
    wqT_ext = nc.declare_dram_parameter("wqT", [D, HD], dt.float16, isOutput=False)
    wkT_ext = nc.declare_dram_parameter("wkT", [D, HD], dt.float16, isOutput=False)
    wvT_ext = nc.declare_dram_parameter("wvT", [D, HD], dt.float16, isOutput=False)
    wpT_ext = nc.declare_dram_parameter("wpT", [D, D], dt.float16, isOutput=False)
    bias_ext = nc.declare_dram_parameter("bias", [1, D], dt.float16, isOutput=False)
    out_ext = nc.declare_dram_parameter("out", [TS, D], dt.float16, isOutput=True)

    with tile.TileContext(nc) as tc, ExitStack() as ctx:
        singles = ctx.enter_context(tc.tile_pool(name="singles", bufs=1))
        dram = ctx.enter_context(tc.tile_pool(name="dram", bufs=4, space="DRAM"))
        cpool = ctx.enter_context(tc.tile_pool(name="cpool", bufs=4))

        # ---------- persistent SBUF ----------
        xT_k = [
            singles.tile([128, T], dt.float16, tag=f"xT{k}", name=f"xT{k}")
            for k in range(KC)
        ]
        wqT_sb = singles.tile([128, KC, HD], dt.float16)
        wkT_sb = singles.tile([128, KC, HD], dt.float16)
        wvT_sb = singles.tile([128, KC, HD], dt.float16)
        wpT_sb = singles.tile([128, KC, D], dt.float16)
        bias_sb = singles.tile([1, D], dt.float16)
        ones_sb = singles.tile([1, 128], dt.float16)
        ones65f = singles.tile([1, 1 + HD], dt.float32)
        kT_t = [
            singles.tile([128, 512], dt.float16, tag=f"kT{i}", name=f"kT{i}")
            for i in range(NQC)
        ]
        qT_t = [
            singles.tile([128, 512], dt.float16, tag=f"qT{i}", name=f"qT{i}")
            for i in range(NQC)
        ]
        vp_t = [
            singles.tile([128, 1 + HD], dt.float16, tag=f"vp{t}", name=f"vp{t}")
            for t in range(T // 128)
        ]
        outTall_sb = singles.tile([128, KC, TS], dt.float16)

        # A2A pieces: P0 after qc3 (512 tok/dest), P1 after qc5 (256),
        # P2 after qc7 (256). dim0 = dest core for in, src core for out.
        a2a_in = [
            dram.tile([N_CORES, HD, n], dt.float16, tag=f"a2a_in{p}", bufs=1,
                      name=f"a2a_in{p}")
            for p, n in ((0, 512), (1, 256), (2, 128), (3, 128))
        ]
        a2a_out = [
            dram.tile([N_CORES, HD, n], dt.float16, tag=f"a2a_out{p}", bufs=1,
                      name=f"a2a_out{p}")
            for p, n in ((0, 512), (1, 256), (2, 128), (3, 128))
        ]
        # outTall col ranges fed by each piece
        piece_cols = [(0, 512), (512, 768), (768, 896), (896, 1024)]

        # exp table warm-up: a dummy activation with no data deps loads the
        # ACT table set (~2.7us) during the DMA window instead of at the
        # first real exp.
        warm_in = singles.tile([1, 16], dt.float32)
        warm_out = singles.tile([1, 16], dt.float16)
        nc.vector.memset(warm_in[:], 0.0)
        nc.scalar.activation(
            warm_out[:], warm_in[:], mybir.ActivationFunctionType.Exp
        )

        # ---------- weights + constants ----------
        # qk weights first (they gate the first production chains), then
        # the first two xT blocks, then the rest of the weights.
        for w_sb, w_ext in ((wkT_sb, wkT_ext), (wqT_sb, wqT_ext)):
            nc.sync.dma_start(
                out=w_sb[:], in_=w_ext[:].rearrange("(k p) c -> p k c", p=128)
            )
        nc.vector.memset(ones_sb[:], 1.0)
        nc.vector.memset(ones65f[:], 1.0)
        for t in range(T // 128):
            nc.vector.memset(vp_t[t][:, 0:1], 1.0)

        # ---------- xT DMA, token-major ----------
        # Issue instructions cost ~0.6us each on an engine queue, so split
        # them across the Sync and (idle) GpSimd queues to double the
        # issue rate. blocks 0-1: half-partition pieces (64KB) so block 0
        # lands ~4us after issue. blocks 2-7: one DMA per (blk, k)
        # covering both batches via a strided free dim.
        for blk in range(2):
            for b in range(B):
                for k in range(KC):
                    c0 = b * N + blk * 512
                    for ph in range(2):
                        p0 = ph * 64
                        nc.sync.dma_start(
                            out=xT_k[k][p0:p0 + 64, c0:c0 + 512],
                            in_=xT_ext[k * 128 + p0:k * 128 + p0 + 64,
                                       c0:c0 + 512],
                        )
        nc.sync.dma_start(
            out=wvT_sb[:], in_=wvT_ext[:].rearrange("(k p) c -> p k c", p=128)
        )
        nc.sync.dma_start(
            out=wpT_sb[:], in_=wpT_ext[:].rearrange("(k p) c -> p k c", p=128)
        )
        nc.sync.dma_start(out=bias_sb[:], in_=bias_ext[:])
        for blk in range(2, NQC):
            for k in range(KC):
                eng = nc.sync
                t_ap = xT_k[k][:]
                e_ap = xT_ext[k * 128:(k + 1) * 128, :]
                off = blk * 512
                eng.dma_start(
                    out=bass.AP(
                        tensor=t_ap.tensor, offset=t_ap.offset + off,
                        ap=[list(t_ap.ap[0]), [N, B], [1, 512]],
                    ),
                    in_=bass.AP(
                        tensor=e_ap.tensor, offset=e_ap.offset + off,
                        ap=[list(e_ap.ap[0]), [N, B], [1, 512]],
                    ),
                )

        # ---------- attention ----------
        with (
            tc.tile_pool(name="pst", bufs=2, space="PSUM") as pst,
            tc.tile_pool(name="pacc", bufs=4, space="PSUM") as pacc,
        ):
            def produce_kq(w_sb, dst, blk, pname):
                ps = pacc.tile([128, 512], dt.float32, tag="acc", name=f"{pname}{blk}")
                for k in range(KC):
                    nc.tensor.matmul(
                        ps[0:64, :],
                        lhsT=w_sb[:, k, :],
                        rhs=xT_k[k][:, blk * 512:(blk + 1) * 512],
                        start=(k == 0), stop=(k == KC - 1),
                        tile_position=(0, 0),
                    )
                    nc.tensor.matmul(
                        ps[64:128, :],
                        lhsT=w_sb[:, k, :],
                        rhs=xT_k[k][:, N + blk * 512:N + (blk + 1) * 512],
                        start=(k == 0), stop=(k == KC - 1),
                        tile_position=(0, 64),
                    )
                nc.vector.tensor_copy(dst[:], ps[:])

            def produce_v(t):
                pv = pacc.tile([128, HD], dt.float32, tag="acc", name=f"pv{t}")
                for k in range(KC):
                    nc.tensor.matmul(
                        pv[:],
                        lhsT=xT_k[k][:, t * 128:(t + 1) * 128],
                        rhs=wvT_sb[:, k, :],
                        start=(k == 0), stop=(k == KC - 1),
                    )
                nc.vector.tensor_copy(vp_t[t][:, 1:1 + HD], pv[:])

            def emit_scores_exp(qc, kt):
                st = pst.tile([128, B, QC], dt.float32, tag="st",
                              name=f"st{qc}_{kt}")
                for pair in range(B):
                    pb = pair * 64
                    lhs_k = kT_t[kt // 4][pb:pb + 64,
                                          (kt % 4) * 128:(kt % 4) * 128 + 128]
                    nc.tensor.matmul(
                        st[:, pair, :],
                        lhsT=lhs_k,
                        rhs=qT_t[qc][pb:pb + 64, :],
                        start=True,
                        stop=True,
                        tile_position=(pb, 0),
                    )
                e = cpool.tile([128, B, QC], dt.float16, tag="e", bufs=26,
                               name=f"e{qc}_{kt}")
                nc.scalar.activation(
                    e[:], st[:], mybir.ActivationFunctionType.Exp, scale=SCALE
                )
                return e

            # normalization: reciprocal of the denominator row, partition
            # broadcast, fused scale-multiply, A2A slice scatter. Pairs are
            # interleaved to halve the serial latency. qc0-6 broadcast via
            # a DRAM bounce (off every engine); qc7 — the exposed tail —
            # broadcasts via a K=1 PE matmul instead (no DMA hops; the
            # PSUM ring is free by then).
            def emit_norm(qc, accs):
                rvs = []
                for pair in range(B):
                    rvec = cpool.tile([1, QC], dt.float32, tag="rvec",
                                      name=f"rv{qc}_{pair}")
                    nc.vector.reciprocal(rvec[:], accs[pair][0:1, :])
                    rvs.append(rvec)
                bcs = []
                if qc < NQC - 1:
                    rds = []
                    for pair in range(B):
                        rdram = dram.tile([1, QC], dt.float32, tag="rdram")
                        nc.sync.dma_start(out=rdram[:], in_=rvs[pair][:])
                        rds.append(rdram)
                    for pair in range(B):
                        bcast = cpool.tile([1 + HD, QC], dt.float32,
                                           tag="bcast")
                        r_ap = rds[pair][:]
                        nc.sync.dma_start(
                            out=bcast[:],
                            in_=bass.AP(
                                tensor=r_ap.tensor, offset=r_ap.offset,
                                ap=[[0, 1 + HD]] + list(r_ap.ap[1:]),
                            ),
                        )
                        bcs.append((bcast, accs[pair]))
                else:
                    for pair in range(B):
                        bc = pacc.tile([1 + HD, QC], dt.float32, tag="acc",
                                       name=f"bc{qc}_{pair}")
                        nc.tensor.matmul(
                            bc[:], lhsT=ones65f[:], rhs=rvs[pair][:],
                            start=True, stop=True,
                        )
                        accS = cpool.tile([1 + HD, QC], dt.float32,
                                          tag="bcast", name=f"aS{qc}_{pair}")
                        nc.vector.tensor_copy(accS[:], accs[pair][:])
                        bcs.append((bc, accS))
                for pair in range(B):
                    src_a, src_b = bcs[pair]
                    outTn = cpool.tile([1 + HD, QC], dt.float16, tag="outTn",
                                       name=f"oTn{qc}_{pair}")
                    nc.vector.tensor_mul(outTn[:], src_b[:], src_a[:])
                    # scatter into the A2A piece buffers
                    if qc < 4:
                        j0 = qc * 4 + pair * 2
                        for h in range(2):
                            j = j0 + h
                            dest, pos = j % N_CORES, j // N_CORES
                            nc.sync.dma_start(
                                out=a2a_in[0][dest][:,
                                                    pos * 256:pos * 256 + 256],
                                in_=outTn[1:1 + HD, h * 256:h * 256 + 256],
                            )
                    elif qc < 6:
                        for h in range(2):
                            j = (qc - 4) * 4 + pair * 2 + h
                            nc.sync.dma_start(
                                out=a2a_in[1][j][:, :],
                                in_=outTn[1:1 + HD, h * 256:h * 256 + 256],
                            )
                    else:
                        # split the last pieces' writes across two queues:
                        # 8 issue instructions on one queue cost ~4.4us of
                        # exposed tail latency otherwise
                        piece = 2 if qc == 6 else 3
                        eng = nc.sync if pair == 0 else nc.gpsimd
                        for h4 in range(4):
                            j = pair * 4 + h4
                            eng.dma_start(
                                out=a2a_in[piece][j][:, :],
                                in_=outTn[1:1 + HD, h4 * 128:h4 * 128 + 128],
                            )
                if qc == 3:
                    trigger_piece(0)
                elif qc == 5:
                    trigger_piece(1)
                elif qc == 6:
                    trigger_piece(2)
                elif qc == 7:
                    trigger_piece(3)

            def trigger_piece(p):
                nc.gpsimd.collective_compute(
                    "AllToAll",
                    mybir.AluOpType.bypass,
                    replica_groups=[list(range(N_CORES))],
                    ins=[a2a_in[p].opt()],
                    outs=[a2a_out[p].opt()],
                )

            def unpack_piece(p):
                lo, hi = piece_cols[p]
                for k in range(KC):
                    nc.gpsimd.dma_start(
                        out=outTall_sb[:, k, lo:hi],
                        in_=a2a_out[p][2 * k:2 * k + 2].rearrange(
                            "a d n -> (a d) n"),
                    )

            def proj_subtile(ts_i):
                yp = pacc.tile([128, D], dt.float32, tag="acc", name=f"yp{ts_i}")
                for k in range(KC):
                    nc.tensor.matmul(
                        yp[:],
                        lhsT=outTall_sb[:, k, ts_i * 128:(ts_i + 1) * 128],
                        rhs=wpT_sb[:, k, :],
                        start=(k == 0),
                        stop=False,
                    )
                nc.tensor.matmul(
                    yp[:],
                    lhsT=ones_sb[:],
                    rhs=bias_sb[:],
                    start=False,
                    stop=True,
                )
                y_sb = cpool.tile([128, D], dt.float16, tag="y", name=f"y{ts_i}")
                nc.vector.tensor_copy(y_sb[:], yp[:])
                # split across two queues so the final output lands faster
                for ph in range(2):
                    p0 = ph * 64
                    nc.sync.dma_start(
                        out=out_ext[ts_i * 128 + p0:ts_i * 128 + p0 + 64, :],
                        in_=y_sb[p0:p0 + 64, :],
                    )

            # pre-loop production (gated on block-0/1 DMA + weights)
            produce_kq(wkT_sb, kT_t[0], 0, "k")
            produce_kq(wqT_sb, qT_t[0], 0, "q")
            produce_kq(wkT_sb, kT_t[1], 1, "k")

            # JIT production schedule: extras[(qc, kt)] emitted right after
            # that slot's exp.
            extras = {}
            for blk in range(2, NQC):
                extras.setdefault((0, 4 * blk - 6), []).append(
                    (lambda b: lambda: produce_kq(wkT_sb, kT_t[b], b, "k"))(blk))
            extras.setdefault((0, 26), []).append(
                lambda: produce_kq(wqT_sb, qT_t[1], 1, "q"))
            for i, n_ in enumerate(range(2, 6)):
                extras.setdefault((1, 2 + 8 * i), []).append(
                    (lambda m: lambda: produce_kq(wqT_sb, qT_t[m], m, "q"))(n_))
            for i, n_ in enumerate(range(6, NQC)):
                extras.setdefault((2, 2 + 8 * i), []).append(
                    (lambda m: lambda: produce_kq(wqT_sb, qT_t[m], m, "q"))(n_))

            # PV stream state
            pending = deque()   # (qc, kt, e_tile)
            acc_of = {}         # qc -> [acc_b0, acc_b1]
            v_done = set()
            pv_emitted = 0

            def emit_pv_pair():
                qc, kt, e = pending.popleft()
                if qc not in acc_of:
                    acc_of[qc] = [
                        pacc.tile([1 + HD, QC], dt.float32, tag="acc",
                                  name=f"acc{qc}_{p}")
                        for p in range(B)
                    ]
                for pair in range(B):
                    vidx = pair * NKT + kt
                    if vidx not in v_done:
                        produce_v(vidx)
                        v_done.add(vidx)
                    nc.tensor.matmul(
                        acc_of[qc][pair][:, :],
                        lhsT=vp_t[vidx][:],
                        rhs=e[:, pair, :],
                        start=(kt == 0),
                        stop=(kt == NKT - 1),
                    )
                if kt == NKT - 1:
                    emit_norm(qc, acc_of.pop(qc))

            for qc in range(NQC):
                base = pv_emitted
                for kt in range(NKT):
                    e = emit_scores_exp(qc, kt)
                    pending.append((qc, kt, e))
                    for fn in extras.get((qc, kt), ()):
                        fn()
                    # pace PV emission: quota spread evenly across the qc
                    goal = base + (PV_QUOTA[qc] * (kt + 1)) // NKT
                    while pv_emitted < goal and pending:
                        emit_pv_pair()
                        pv_emitted += 1
                    # late-attention overlap of unpack + proj for the
                    # already-landed pieces: one subtile every other slot so
                    # the PE burst never outruns the pst-buffered exp stream
                    if qc == 7:
                        # start at kt=4: qc6's accs (norm runs ~2 slots in
                        # due to the boundary lag) must free their ring
                        # slots before proj's PSUM allocations
                        if 4 <= kt <= 14 and kt % 2 == 0:
                            proj_subtile(kt // 2 - 2)
                        elif kt == 24:
                            unpack_piece(2)
                            proj_subtile(6)
                # P0 and P1 are both complete well before qc6 ends
                if qc == 6:
                    unpack_piece(0)
                    unpack_piece(1)

            while pending:
                emit_pv_pair()
                pv_emitted += 1

            # keep the PE HAM-warm through the last A2A flight so the final
            # proj subtile runs at full clock
            scratch = pacc.tile([1, D], dt.float32, tag="acc", name="scratch")
            for _ in range(36):
                nc.tensor.matmul(
                    scratch[:], lhsT=ones_sb[:, 0:1], rhs=bias_sb[:],
                    start=True, stop=True,
                )
            unpack_piece(3)
            proj_subtile(7)

    return nc


def _get_nc():
    if "nc" not in _COMPILED:
        _COMPILED["nc"] = _build()
    return _COMPILED["nc"]


def _seg_token(piece, j):
    """Map (piece, subchunk index) -> (batch, token start, length)."""
    if piece == 0:
        qc = j // 4
        rem = j % 4
        return rem // 2, qc * 512 + (rem % 2) * 256, 256
    if piece == 1:
        qc = 4 + j // 4
        rem = j % 4
        return rem // 2, qc * 512 + (rem % 2) * 256, 256
    qc = 6 if piece == 2 else 7
    return j // 4, qc * 512 + (j % 4) * 128, 128


def kernel(x, w_qkv, w_proj, b_proj):
    from concourse.bass_utils import run_bass_kernel_spmd

    x = np.asarray(x, dtype=np.float32)
    w_qkv = np.asarray(w_qkv, dtype=np.float32)
    w_proj = np.asarray(w_proj, dtype=np.float32)
    b_proj = np.asarray(b_proj, dtype=np.float32)

    xT = np.ascontiguousarray(x.transpose(2, 0, 1).reshape(D, T)).astype(F16)
    wpT = np.ascontiguousarray(w_proj.T).astype(F16)
    bias = b_proj.reshape(1, D).astype(F16)

    in_maps = []
    for d in range(N_CORES):
        wq = w_qkv[0 * D + d * HD: 0 * D + (d + 1) * HD, :]
        wk = w_qkv[1 * D + d * HD: 1 * D + (d + 1) * HD, :]
        wv = w_qkv[2 * D + d * HD: 2 * D + (d + 1) * HD, :]
        in_maps.append({
            "xT": xT,
            "wqT": np.ascontiguousarray(wq.T).astype(F16),
            "wkT": np.ascontiguousarray(wk.T).astype(F16),
            "wvT": np.ascontiguousarray(wv.T).astype(F16),
            "wpT": wpT,
            "bias": bias,
        })

    nc = _get_nc()
    res = run_bass_kernel_spmd(nc, in_maps, core_ids=list(range(N_CORES)))

    y = np.empty((B, N, D), dtype=np.float32)
    for s in range(N_CORES):
        r = np.asarray(res.results[s]["out"], dtype=np.float32)
        segs = [(0, s), (0, s + 8), (1, s), (2, s), (3, s)]
        row = 0
        for piece, j in segs:
            b, t0, ln = _seg_token(piece, j)
            y[b, t0:t0 + ln, :] = r[row:row + ln, :]
            row += ln
    return y
